# revision 38
# baseline (speedup 1.0000x reference)
"""DAGCN kernel v5 for Trainium2, 8 NeuronCores, sharded over T (3 t/core).

Math per t (N=512 nodes, C=O=64, B=32, K=3):
  A    = relu(E E^T)  (rank-1 outer product, symmetric)
  PU   = exp(A) = max(exp(e_n e_m), 1)   (symmetric)
  Z_n  = sum_m PU[m, n]  (column sums == row sums by symmetry)
  S    = PU / Z  (row softmax);  d_n = S[n,n] = exp(e_n^2)/Z_n
  xg1  = S @ x;  xg2 = 2 d xg1 - x
  out  = x W0 + xg1 W1 + xg2 W2 + bias
       = x (W0 - W2) + xg1 W1 + (2 d xg1) W2 + bias     <- regrouped

Key structure vs the v3 baseline (185us):
  - Algebraic regroup removes the second message pass entirely: only
    xg1 = S@x is computed on the PE (64 vs 128 big matmuls per t), and
    xg1d = 2d*xg1 is a cheap elementwise multiply of the same psum.
  - No PE transposes for scores: PU is symmetric, so the e1T moving
    operand srt[m,n] = PU[m,n] * inv[n] is a column-scale of the PU tile.
    Column broadcasts (inv, 2d) are materialized as [128, N] tiles via a
    k=1 ones-row matmul (the PE is the only partition broadcaster).
  - Column sums via k=1 matmul with a ones column (PE, not DVE).
  - Weights quantized by numeric class: M2 weights [W1; W2] ship as
    fp8 e3m4 (they only multiply the small xg1/xg1d terms; measured
    absmax-rel ~1.3e-2 total), while [W0-W2; bias] stays bf16 (it
    multiplies x, 99.5% of output variance). Weight DMA drops from
    12.06 to 8.06 MiB/t. W0-W2 is precombined on the host.
  - Final contraction per n: 2 matmuls (k=65 bf16, k=128 with fp8
    moving), psum [32b, 8n, 64o] per group, contiguous [b, n, o] store.
"""
import sys

sys.path.insert(0, "/opt/trn_rl_repo")
import numpy as np

CFG = ""

B, T, N, C, O, K = 32, 24, 512, 64, 64, 3
NCORES = 8
T_LOC = T // NCORES  # 3 time steps per core

_CACHE = {}


def build_bass(reps=1):
    if ("nc", reps) in _CACHE:
        return _CACHE[("nc", reps)]
    from contextlib import ExitStack

    import concourse.mybir as mybir
    from concourse import bacc
    import concourse.tile as tile
    from concourse.bass import ts

    f32 = mybir.dt.float32
    f32r = mybir.dt.float32r
    bf16 = mybir.dt.bfloat16
    f8e3 = mybir.dt.float8e3
    Alu = mybir.AluOpType
    Act = mybir.ActivationFunctionType

    nc = bacc.Bacc()
    xm_d = nc.dram_tensor("xmbc_sh", [T_LOC, N, B, C], bf16, kind="ExternalInput")
    xt_d = nc.dram_tensor("xT_sh", [T_LOC, C, B, N], bf16, kind="ExternalInput")
    e_d = nc.dram_tensor("emb_sh", [T_LOC, N], f32r, kind="ExternalInput")
    w0b_d = nc.dram_tensor("w0b_sh", [T_LOC, C + 1, N, O], bf16, kind="ExternalInput")
    w12_d = nc.dram_tensor("w12_sh", [T_LOC, 2 * C, N, O], f8e3, kind="ExternalInput")
    on_d = nc.dram_tensor("ones_sh", [1, B * N], bf16, kind="ExternalInput")
    # out packed for 128-partition DMA: [t, hc, g, b, h, n8, o]
    # n = hc*64 + h*32 + g*8 + n8; partitions = (g, b)
    o_d = nc.dram_tensor("out_sh", [T_LOC, 8, 128, 2, 8, O], bf16,
                         kind="ExternalOutput")

    with tile.TileContext(nc) as tc, ExitStack() as ctx:
        p1 = ctx.enter_context(tc.tile_pool(name="singles", bufs=1))
        p_row = ctx.enter_context(tc.tile_pool(name="rows", bufs=1))
        p_bc = ctx.enter_context(tc.tile_pool(name="bcast", bufs=2))
        p_pu = ctx.enter_context(tc.tile_pool(name="pu", bufs=4))
        p_srt = ctx.enter_context(tc.tile_pool(name="srt", bufs=5))
        p_x = ctx.enter_context(tc.tile_pool(name="xin", bufs=2))
        p_w0 = ctx.enter_context(tc.tile_pool(name="w0", bufs=4))
        p_w12 = ctx.enter_context(tc.tile_pool(name="w12", bufs=3))
        p_ob = ctx.enter_context(tc.tile_pool(name="ob", bufs=2))
        p_ps = ctx.enter_context(tc.tile_pool(name="ps", bufs=4, space="PSUM"))
        p_pso = ctx.enter_context(tc.tile_pool(name="pso", bufs=2, space="PSUM"))
        p_psob = ctx.enter_context(tc.tile_pool(name="psob", bufs=2, space="PSUM"))

        # persistent stacks:
        #   stack1 [65, B, N]  rows 0:64 x^T (c,b,n), row 64 ones (bias lane)
        #     double-buffered by t parity so next-t x loads never wait on the
        #     current final phase still reading it
        #   stack2 [128, B, N] rows 0:64 xg1^T, rows 64:128 xg1d^T
        stack1s = [p1.tile([C + 1, B, N], bf16, name=f"stk1_{i}", tag=f"s1_{i}")
                   for i in range(2)]
        stack2 = p1.tile([2 * C, B, N], bf16, name="stk2", tag="s2")
        # ones column [128, 1] for column-sum matmuls (k=1 trick needs a
        # [1, 128] stationary; ones row slice of on_d serves both)
        ones_col = p1.tile([128, 1], bf16, name="onec", tag="oc")
        nc.gpsimd.dma_start(out=ones_col, in_=on_d[0, 0:128].rearrange("(p f) -> p f", f=1))
        ones_row = p1.tile([1, 128], bf16, name="oner", tag="or")
        nc.gpsimd.dma_start(out=ones_row, in_=on_d[:, 0:128])

        def emit_ones_init():
            # stack1 row 64 = ones: split the [1, 32KB] single-partition DMA
            # into quarters on different queues; emitted after the critical
            # startup loads (only needed by the first final phase ~25us in)
            for i in range(2):
                for q, qeng in enumerate([nc.sync, nc.scalar, nc.gpsimd, nc.sync]):
                    qeng.dma_start(
                        out=stack1s[i][C:C + 1, 8 * q:8 * (q + 1), :].rearrange(
                            "p b n -> p (b n)"),
                        in_=on_d[:, 4096 * q:4096 * (q + 1)])

        # ---- software-pipelined emission ----
        # scores(t+1) is computed during final(t): PU matmuls right after
        # e1T(t); the cheap zs/bcast matmuls interleave between final-phase
        # hc chunks so the PE queue never stalls at a dependency head.
        PG = 8

        def emit_loads(t, tt):
            st = {"t": t, "tt": tt}
            e_row = p_row.tile([1, N], f32r, tag="erow")
            nc.scalar.dma_start(out=e_row, in_=e_d[t][None, :])
            st["e_row"] = e_row
            xmall = p_x.tile([128, 4, B, C], bf16, tag="xm")
            nc.sync.dma_start(
                out=xmall,
                in_=xm_d[t].rearrange("(mc p) b c -> p mc b c", p=128))
            st["xmall"] = xmall
            return st

        def emit_scoresA(st):
            e_row = st["e_row"]
            pus = []
            for mc in range(4):
                ps = p_ps.tile([128, N], f32, tag="big")
                nc.tensor.matmul(ps[:], e_row[:, ts(mc, 128)], e_row[:],
                                 start=True, stop=True)
                pu = p_pu.tile([128, N], bf16, tag="pu")
                nc.scalar.activation(pu[:], ps[:], Act.Exp)
                nc.vector.tensor_single_scalar(pu[:], pu[:], 1.0, Alu.max)
                pus.append(pu)
            st["pus"] = pus
            # row ops that only need e_row
            sq = p_row.tile([1, N], f32, tag="sq")
            nc.vector.tensor_mul(sq[:], e_row[:], e_row[:])
            esq = p_row.tile([1, N], f32, tag="esq")
            nc.scalar.activation(esq[:], sq[:], Act.Exp)
            st["esq"] = esq

        def emit_zs(st):
            zs_ps = p_ps.tile([128, N], f32, tag="big")
            for mc in range(4):
                nc.tensor.matmul(zs_ps[0:1, :], ones_col[:], st["pus"][mc][:],
                                 start=(mc == 0), stop=(mc == 3))
            st["zs_ps"] = zs_ps

        def emit_invd2(st):
            inv_row = p_row.tile([1, N], bf16, tag="invr")
            with nc.allow_low_precision(reason="inv feeds bf16 bcast matmul"):
                nc.vector.reciprocal(inv_row[:], st["zs_ps"][0:1, :])
            st["inv_row"] = inv_row
            t1 = p_row.tile([1, N], f32, tag="t1")
            nc.vector.tensor_tensor(out=t1[:], in0=st["esq"][:],
                                    in1=inv_row[:], op=Alu.mult)
            d2_row = p_row.tile([1, N], bf16, tag="d2r")
            nc.vector.tensor_single_scalar(d2_row[:], t1[:], 2.0, Alu.mult)
            st["d2_row"] = d2_row

        def emit_bcasts(st):
            invb_ps = p_ps.tile([128, N], f32, tag="big")
            nc.tensor.matmul(invb_ps[:], ones_row[:], st["inv_row"][:],
                             start=True, stop=True)
            inv_bc = p_bc.tile([128, N], bf16, tag="invbc")
            nc.vector.tensor_copy(out=inv_bc[:], in_=invb_ps[:])
            st["inv_bc"] = inv_bc
            d2b_ps = p_ps.tile([128, N], f32, tag="big")
            nc.tensor.matmul(d2b_ps[:], ones_row[:], st["d2_row"][:],
                             start=True, stop=True)
            d2_bc = p_bc.tile([128, N], bf16, tag="d2bc")
            nc.vector.tensor_copy(out=d2_bc[:], in_=d2b_ps[:])
            st["d2_bc"] = d2_bc

        def emit_srts(st):
            srts = []
            for mc in range(4):
                srt = p_srt.tile([128, N], bf16, tag="srt")
                nc.vector.tensor_tensor(out=srt[:], in0=st["pus"][mc][:],
                                        in1=st["inv_bc"][:], op=Alu.mult)
                srts.append(srt)
            st["srts"] = srts

        def emit_xt_chunk(st, j):
            # direct [64, 4KB] DMA of x^T rows for b-quarter j into the
            # parity stack, on the scalar queue where stores leave slack
            stk = stack1s[st["tt"] % 2]
            nc.scalar.dma_start(out=stk[0:C, 4 * j:4 * (j + 1), :],
                                in_=xt_d[st["t"], :, 4 * j:4 * (j + 1), :])

        def emit_e1T(st):
            xmall, srts, d2_bc = st["xmall"], st["srts"], st["d2_bc"]
            for pr in range(16):  # b-pairs
                b0 = 2 * pr
                ps1 = p_ps.tile([128, N], f32, tag="big")
                for mc in range(4):
                    lhs = xmall[:, mc, b0:b0 + 2, :].rearrange("p b c -> p (b c)")
                    nc.tensor.matmul(ps1[:], lhs, srts[mc][:],
                                     start=(mc == 0), stop=(mc == 3))
                # xg1 psum->sbuf copies on DVE/Act (gpsimd cannot read PSUM);
                # xg1d = xg1 * d2 as sbuf-only TT on Pool engine
                ceng = [nc.scalar, nc.vector][pr % 2]
                if ceng is nc.scalar:
                    ceng.copy(out=stack2[0:C, b0, :], in_=ps1[0:C])
                else:
                    ceng.tensor_copy(out=stack2[0:C, b0, :], in_=ps1[0:C])
                ceng2 = [nc.vector, nc.scalar][pr % 2]
                if ceng2 is nc.scalar:
                    ceng2.copy(out=stack2[0:C, b0 + 1, :], in_=ps1[C:])
                else:
                    ceng2.tensor_copy(out=stack2[0:C, b0 + 1, :], in_=ps1[C:])
                nc.gpsimd.tensor_tensor(
                    out=stack2[C:, b0, :], in0=stack2[0:C, b0, :],
                    in1=st["d2_bc"][0:C], op=Alu.mult)
                nc.gpsimd.tensor_tensor(
                    out=stack2[C:, b0 + 1, :], in0=stack2[0:C, b0 + 1, :],
                    in1=st["d2_bc"][0:C], op=Alu.mult)

        def emit_wload(t, tt, hc):
            w0t = p_w0.tile([C + 1, 64, O], bf16, tag="w0", name=f"w0t_{tt}_{hc}")
            w12t = p_w12.tile([2 * C, 64, O], f8e3, tag="w12", name=f"w12t_{tt}_{hc}")
            nc.sync.dma_start(out=w0t, in_=w0b_d[t, :, ts(hc, 64), :])
            nc.gpsimd.dma_start(out=w12t, in_=w12_d[t, :, ts(hc, 64), :])
            return w0t, w12t

        def emit_final_chunk(t, tt, hc, w0t, w12t):
            # psum packed as 4 bands of 32 partitions; band 96 is illegal for
            # matmul (quadrant-3 HW bug): bands 0..2 in ps_a, band 3 in ps_b;
            # the copies shift band 3 to sbuf partitions 96:128.
            out_sb = p_ob.tile([128, 2, PG, O], bf16, tag="osb")
            for h in range(2):  # 32-n halves
                ps_a = p_pso.tile([96, PG, O], f32, tag="poa")
                ps_b = p_psob.tile([32, PG, O], f32, tag="pob")
                for g in range(4):  # partition bands
                    for j in range(PG):
                        nl = h * 32 + g * PG + j
                        ng = hc * 64 + nl
                        dst_ps = (ps_a[32 * g:32 * (g + 1), j, :]
                                  if g < 3 else ps_b[:, j, :])
                        nc.tensor.matmul(
                            dst_ps, stack1s[tt % 2][:, :, ng],
                            w0t[:, nl, :], start=True, stop=False)
                        nc.tensor.matmul(
                            dst_ps, stack2[:, :, ng],
                            w12t[:, nl, :], start=False, stop=True)
                dsta = out_sb[0:96, h, :, :].rearrange("p n o -> p (n o)")
                dstb = out_sb[96:128, h, :, :].rearrange("p n o -> p (n o)")
                nc.vector.tensor_copy(
                    out=dsta, in_=ps_a[:].rearrange("p n o -> p (n o)"))
                nc.scalar.copy(
                    out=dstb, in_=ps_b[:].rearrange("p n o -> p (n o)"))
            nc.scalar.dma_start(out=o_d[t, hc], in_=out_sb[:])

        NT = T_LOC * reps
        st = emit_loads(0, 0)
        for j in range(8):
            emit_xt_chunk(st, j)
        emit_scoresA(st)
        emit_ones_init()
        emit_zs(st)
        emit_invd2(st)
        emit_bcasts(st)
        emit_srts(st)
        for tt in range(NT):
            t = tt % T_LOC
            emit_e1T(st)
            # prefetch first 4 weight chunks before next-t input loads so the
            # final phase is never starved behind them on the queues
            wts = {hc: emit_wload(t, tt, hc) for hc in range(3)}
            nxt = None
            if tt + 1 < NT:
                nxt = emit_loads((tt + 1) % T_LOC, tt + 1)
                emit_scoresA(nxt)
            for hc in range(8):
                emit_final_chunk(t, tt, hc, *wts.pop(hc))
                if hc + 3 < 8:
                    wts[hc + 3] = emit_wload(t, tt, hc + 3)
                if nxt is not None:
                    emit_xt_chunk(nxt, hc)
                    if hc == 0:
                        emit_zs(nxt)
                    elif hc == 1:
                        emit_invd2(nxt)
                    elif hc == 3:
                        emit_bcasts(nxt)
                    elif hc == 5:
                        emit_srts(nxt)
            st = nxt

    nc.finalize()
    _CACHE[("nc", reps)] = nc
    return nc


def make_in_maps(inputs):
    import ml_dtypes
    bf16 = ml_dtypes.bfloat16
    f8e3 = ml_dtypes.float8_e3m4

    x = np.asarray(inputs["x"], dtype=np.float32)
    emb = np.asarray(inputs["dn_embeddings"], dtype=np.float32)
    w = np.asarray(inputs["weights_pool"], dtype=np.float32)
    bias = np.asarray(inputs["bias_pool"], dtype=np.float32)

    in_maps = []
    for c in range(NCORES):
        sl = slice(c * T_LOC, (c + 1) * T_LOC)
        xs = x[:, sl]  # [B, T_LOC, N, C]
        ws = w[sl]  # [T_LOC, N, K, C, O]
        # w0b rows 0:64 = (W0 - W2)^T(i,n,o), row 64 = bias
        w0b = np.empty((T_LOC, C + 1, N, O), np.float32)
        w0b[:, 0:C] = (ws[:, :, 0] - ws[:, :, 2]).transpose(0, 2, 1, 3)
        w0b[:, C] = bias[sl]
        # w12 rows 0:64 = W1^T, rows 64:128 = W2^T
        w12 = np.concatenate([
            ws[:, :, 1].transpose(0, 2, 1, 3),
            ws[:, :, 2].transpose(0, 2, 1, 3)], axis=1)
        in_maps.append({
            "xmbc_sh": np.ascontiguousarray(
                xs.transpose(1, 2, 0, 3)).astype(bf16),
            "xT_sh": np.ascontiguousarray(
                xs.transpose(1, 3, 0, 2)).astype(bf16),
            "emb_sh": np.ascontiguousarray(emb[sl]),
            "w0b_sh": np.ascontiguousarray(w0b).astype(bf16),
            "w12_sh": np.ascontiguousarray(w12).astype(f8e3),
            "ones_sh": np.ones((1, B * N), dtype=bf16),
        })
    return in_maps


def run_spmd(inputs, **kwargs):
    from concourse.bass_utils import run_bass_kernel_spmd

    nc = build_bass()
    in_maps = make_in_maps(inputs)
    res = run_bass_kernel_spmd(nc, in_maps, core_ids=list(range(NCORES)), **kwargs)
    outs = []
    for r in res.results:
        o2 = np.asarray(r["out_sh"]).reshape(T_LOC, 8, 4, B, 2, 8, O)
        # n = hc*64 + h*32 + g*8 + n8  ->  [B, T_LOC, hc, h, g, n8, O]
        o = o2.transpose(3, 0, 1, 4, 2, 5, 6).reshape(B, T_LOC, N, O)
        outs.append(o)
    out = np.concatenate(outs, axis=1)
    return out.astype(np.float32), res


def kernel(**inputs):
    out, _ = run_spmd(inputs)
    return out


# revision 41
# speedup vs baseline: 1.4593x; 1.4593x over previous
"""DAGCN kernel v5 for Trainium2, 8 NeuronCores, sharded over T (3 t/core).

Math per t (N=512 nodes, C=O=64, B=32, K=3):
  A    = relu(E E^T)  (rank-1 outer product, symmetric)
  PU   = exp(A) = max(exp(e_n e_m), 1)   (symmetric)
  Z_n  = sum_m PU[m, n]  (column sums == row sums by symmetry)
  S    = PU / Z  (row softmax);  d_n = S[n,n] = exp(e_n^2)/Z_n
  xg1  = S @ x;  xg2 = 2 d xg1 - x
  out  = x W0 + xg1 W1 + xg2 W2 + bias
       = x (W0 - W2) + xg1 W1 + (2 d xg1) W2 + bias     <- regrouped

Key structure vs the v3 baseline (185us):
  - Algebraic regroup removes the second message pass entirely: only
    xg1 = S@x is computed on the PE (64 vs 128 big matmuls per t), and
    xg1d = 2d*xg1 is a cheap elementwise multiply of the same psum.
  - No PE transposes for scores: PU is symmetric, so the e1T moving
    operand srt[m,n] = PU[m,n] * inv[n] is a column-scale of the PU tile.
    Column broadcasts (inv, 2d) are materialized as [128, N] tiles via a
    k=1 ones-row matmul (the PE is the only partition broadcaster).
  - Column sums via k=1 matmul with a ones column (PE, not DVE).
  - Weights quantized by numeric class: M2 weights [W1; W2] ship as
    fp8 e3m4 (they only multiply the small xg1/xg1d terms; measured
    absmax-rel ~1.3e-2 total), while [W0-W2; bias] stays bf16 (it
    multiplies x, 99.5% of output variance). Weight DMA drops from
    12.06 to 8.06 MiB/t. W0-W2 is precombined on the host.
  - Final contraction per n: 2 matmuls (k=65 bf16, k=128 with fp8
    moving), psum [32b, 8n, 64o] per group, contiguous [b, n, o] store.
"""
import sys

sys.path.insert(0, "/opt/trn_rl_repo")
import numpy as np

CFG = ""

B, T, N, C, O, K = 32, 24, 512, 64, 64, 3
NCORES = 8
T_LOC = T // NCORES  # 3 time steps per core

_CACHE = {}


def build_bass(reps=1):
    if ("nc", reps) in _CACHE:
        return _CACHE[("nc", reps)]
    from contextlib import ExitStack

    import concourse.mybir as mybir
    from concourse import bacc
    import concourse.tile as tile
    from concourse.bass import ts

    f32 = mybir.dt.float32
    f32r = mybir.dt.float32r
    bf16 = mybir.dt.bfloat16
    f8e3 = mybir.dt.float8e3
    Alu = mybir.AluOpType
    Act = mybir.ActivationFunctionType

    nc = bacc.Bacc()
    xm_d = nc.dram_tensor("xmbc_sh", [T_LOC, N, B, C], bf16, kind="ExternalInput")
    xt_d = nc.dram_tensor("xT_sh", [T_LOC, C, B, N], bf16, kind="ExternalInput")
    e_d = nc.dram_tensor("emb_sh", [T_LOC, N], f32r, kind="ExternalInput")
    w0b_d = nc.dram_tensor("w0b_sh", [T_LOC, C + 1, N, O], bf16, kind="ExternalInput")
    w12_d = nc.dram_tensor("w12_sh", [T_LOC, 2 * C, N, O], f8e3, kind="ExternalInput")
    on_d = nc.dram_tensor("ones_sh", [1, B * N], bf16, kind="ExternalInput")
    # out packed for 128-partition DMA: [t, hc, g, b, h, n8, o]
    # n = hc*64 + h*32 + g*8 + n8; partitions = (g, b)
    o_d = nc.dram_tensor("out_sh", [T_LOC, 8, 128, 2, 8, O], bf16,
                         kind="ExternalOutput")

    with tile.TileContext(nc) as tc, ExitStack() as ctx:
        p1 = ctx.enter_context(tc.tile_pool(name="singles", bufs=1))
        p_row = ctx.enter_context(tc.tile_pool(name="rows", bufs=1))
        p_bc = ctx.enter_context(tc.tile_pool(name="bcast", bufs=2))
        p_pu = ctx.enter_context(tc.tile_pool(name="pu", bufs=4))
        p_srt = ctx.enter_context(tc.tile_pool(name="srt", bufs=5))
        p_x = ctx.enter_context(tc.tile_pool(name="xin", bufs=2))
        p_w0 = ctx.enter_context(tc.tile_pool(name="w0", bufs=4))
        p_w12 = ctx.enter_context(tc.tile_pool(name="w12", bufs=3))
        p_ob = ctx.enter_context(tc.tile_pool(name="ob", bufs=2))
        p_ps = ctx.enter_context(tc.tile_pool(name="ps", bufs=4, space="PSUM"))
        p_pso = ctx.enter_context(tc.tile_pool(name="pso", bufs=2, space="PSUM"))
        p_psob = ctx.enter_context(tc.tile_pool(name="psob", bufs=2, space="PSUM"))

        # persistent stacks:
        #   stack1 [65, B, N]  rows 0:64 x^T (c,b,n), row 64 ones (bias lane)
        #     double-buffered by t parity so next-t x loads never wait on the
        #     current final phase still reading it
        #   stack2 [128, B, N] rows 0:64 xg1^T, rows 64:128 xg1d^T
        stack1s = [p1.tile([C + 1, B, N], bf16, name=f"stk1_{i}", tag=f"s1_{i}")
                   for i in range(2)]
        stack2 = p1.tile([2 * C, B, N], bf16, name="stk2", tag="s2")
        # ones column [128, 1] for column-sum matmuls (k=1 trick needs a
        # [1, 128] stationary; ones row slice of on_d serves both)
        ones_col = p1.tile([128, 1], bf16, name="onec", tag="oc")
        nc.gpsimd.dma_start(out=ones_col, in_=on_d[0, 0:128].rearrange("(p f) -> p f", f=1))
        ones_row = p1.tile([1, 128], bf16, name="oner", tag="or")
        nc.gpsimd.dma_start(out=ones_row, in_=on_d[:, 0:128])

        def emit_ones_init():
            # stack1 row 64 = ones: split the [1, 32KB] single-partition DMA
            # into quarters on different queues; emitted after the critical
            # startup loads (only needed by the first final phase ~25us in)
            for i in range(2):
                for q, qeng in enumerate([nc.sync, nc.scalar, nc.gpsimd, nc.sync]):
                    qeng.dma_start(
                        out=stack1s[i][C:C + 1, 8 * q:8 * (q + 1), :].rearrange(
                            "p b n -> p (b n)"),
                        in_=on_d[:, 4096 * q:4096 * (q + 1)])

        # ---- software-pipelined emission ----
        # scores(t+1) is computed during final(t): PU matmuls right after
        # e1T(t); the cheap zs/bcast matmuls interleave between final-phase
        # hc chunks so the PE queue never stalls at a dependency head.
        PG = 8

        def emit_loads(t, tt):
            st = {"t": t, "tt": tt}
            e_row = p_row.tile([1, N], f32r, tag="erow")
            nc.scalar.dma_start(out=e_row, in_=e_d[t][None, :])
            st["e_row"] = e_row
            xmall = p_x.tile([128, 4, B, C], bf16, tag="xm")
            nc.sync.dma_start(
                out=xmall,
                in_=xm_d[t].rearrange("(mc p) b c -> p mc b c", p=128))
            st["xmall"] = xmall
            return st

        def emit_scoresA(st):
            e_row = st["e_row"]
            pus = []
            for mc in range(4):
                ps = p_ps.tile([128, N], f32, tag="big")
                nc.tensor.matmul(ps[:], e_row[:, ts(mc, 128)], e_row[:],
                                 start=True, stop=True)
                pu = p_pu.tile([128, N], bf16, tag="pu")
                nc.scalar.activation(pu[:], ps[:], Act.Exp)
                nc.vector.tensor_single_scalar(pu[:], pu[:], 1.0, Alu.max)
                pus.append(pu)
            st["pus"] = pus
            # row ops that only need e_row
            sq = p_row.tile([1, N], f32, tag="sq")
            nc.vector.tensor_mul(sq[:], e_row[:], e_row[:])
            esq = p_row.tile([1, N], f32, tag="esq")
            nc.scalar.activation(esq[:], sq[:], Act.Exp)
            st["esq"] = esq

        def emit_zs(st):
            zs_ps = p_ps.tile([128, N], f32, tag="big")
            for mc in range(4):
                nc.tensor.matmul(zs_ps[0:1, :], ones_col[:], st["pus"][mc][:],
                                 start=(mc == 0), stop=(mc == 3))
            st["zs_ps"] = zs_ps

        def emit_invd2(st):
            inv_row = p_row.tile([1, N], bf16, tag="invr")
            with nc.allow_low_precision(reason="inv feeds bf16 bcast matmul"):
                nc.vector.reciprocal(inv_row[:], st["zs_ps"][0:1, :])
            st["inv_row"] = inv_row
            t1 = p_row.tile([1, N], f32, tag="t1")
            nc.vector.tensor_tensor(out=t1[:], in0=st["esq"][:],
                                    in1=inv_row[:], op=Alu.mult)
            d2_row = p_row.tile([1, N], bf16, tag="d2r")
            nc.vector.tensor_single_scalar(d2_row[:], t1[:], 2.0, Alu.mult)
            st["d2_row"] = d2_row

        def emit_bcasts(st):
            invb_ps = p_ps.tile([128, N], f32, tag="big")
            nc.tensor.matmul(invb_ps[:], ones_row[:], st["inv_row"][:],
                             start=True, stop=True)
            inv_bc = p_bc.tile([128, N], bf16, tag="invbc")
            nc.vector.tensor_copy(out=inv_bc[:], in_=invb_ps[:])
            st["inv_bc"] = inv_bc
            d2b_ps = p_ps.tile([128, N], f32, tag="big")
            nc.tensor.matmul(d2b_ps[:], ones_row[:], st["d2_row"][:],
                             start=True, stop=True)
            d2_bc = p_bc.tile([128, N], bf16, tag="d2bc")
            nc.vector.tensor_copy(out=d2_bc[:], in_=d2b_ps[:])
            st["d2_bc"] = d2_bc

        def emit_srts(st):
            srts = []
            for mc in range(4):
                srt = p_srt.tile([128, N], bf16, tag="srt")
                nc.vector.tensor_tensor(out=srt[:], in0=st["pus"][mc][:],
                                        in1=st["inv_bc"][:], op=Alu.mult)
                srts.append(srt)
            st["srts"] = srts

        def emit_xt_chunk(st, j):
            # direct [64, 4KB] DMA of x^T rows for b-quarter j into the
            # parity stack, on the scalar queue where stores leave slack
            stk = stack1s[st["tt"] % 2]
            nc.scalar.dma_start(out=stk[0:C, 4 * j:4 * (j + 1), :],
                                in_=xt_d[st["t"], :, 4 * j:4 * (j + 1), :])

        def emit_e1T(st):
            xmall, srts = st["xmall"], st["srts"]
            for pr in range(16):  # b-pairs
                b0 = 2 * pr
                ps1 = p_ps.tile([128, N], f32, tag="big")
                for mc in range(4):
                    lhs = xmall[:, mc, b0:b0 + 2, :].rearrange("p b c -> p (b c)")
                    nc.tensor.matmul(ps1[:], lhs, srts[mc][:],
                                     start=(mc == 0), stop=(mc == 3))
                # xg1 psum->sbuf copies on DVE/Act (gpsimd cannot read PSUM);
                # xg1d = xg1 * d2 as sbuf-only TT on Pool engine
                ceng = [nc.scalar, nc.vector][pr % 2]
                if ceng is nc.scalar:
                    ceng.copy(out=stack2[0:C, b0, :], in_=ps1[0:C])
                else:
                    ceng.tensor_copy(out=stack2[0:C, b0, :], in_=ps1[0:C])
                ceng2 = [nc.vector, nc.scalar][pr % 2]
                if ceng2 is nc.scalar:
                    ceng2.copy(out=stack2[0:C, b0 + 1, :], in_=ps1[C:])
                else:
                    ceng2.tensor_copy(out=stack2[0:C, b0 + 1, :], in_=ps1[C:])
                nc.gpsimd.tensor_tensor(
                    out=stack2[C:, b0, :], in0=stack2[0:C, b0, :],
                    in1=st["d2_bc"][0:C], op=Alu.mult)
                nc.gpsimd.tensor_tensor(
                    out=stack2[C:, b0 + 1, :], in0=stack2[0:C, b0 + 1, :],
                    in1=st["d2_bc"][0:C], op=Alu.mult)

        def emit_wload(t, tt, hc):
            w0t = p_w0.tile([C + 1, 64, O], bf16, tag="w0", name=f"w0t_{tt}_{hc}")
            w12t = p_w12.tile([2 * C, 64, O], f8e3, tag="w12", name=f"w12t_{tt}_{hc}")
            nc.sync.dma_start(out=w0t, in_=w0b_d[t, :, ts(hc, 64), :])
            nc.gpsimd.dma_start(out=w12t, in_=w12_d[t, :, ts(hc, 64), :])
            return w0t, w12t

        def emit_final_chunk(t, tt, hc, w0t, w12t):
            # psum packed as 4 bands of 32 partitions; band 96 is illegal for
            # matmul (quadrant-3 HW bug): bands 0..2 in ps_a, band 3 in ps_b;
            # the copies shift band 3 to sbuf partitions 96:128.
            out_sb = p_ob.tile([128, 2, PG, O], bf16, tag="osb")
            for h in range(2):  # 32-n halves
                ps_a = p_pso.tile([96, PG, O], f32, tag="poa")
                ps_b = p_psob.tile([32, PG, O], f32, tag="pob")
                for g in range(4):  # partition bands
                    for j in range(PG):
                        nl = h * 32 + g * PG + j
                        ng = hc * 64 + nl
                        dst_ps = (ps_a[32 * g:32 * (g + 1), j, :]
                                  if g < 3 else ps_b[:, j, :])
                        nc.tensor.matmul(
                            dst_ps, stack1s[tt % 2][:, :, ng],
                            w0t[:, nl, :], start=True, stop=False)
                        nc.tensor.matmul(
                            dst_ps, stack2[:, :, ng],
                            w12t[:, nl, :], start=False, stop=True)
                dsta = out_sb[0:96, h, :, :].rearrange("p n o -> p (n o)")
                dstb = out_sb[96:128, h, :, :].rearrange("p n o -> p (n o)")
                nc.vector.tensor_copy(
                    out=dsta, in_=ps_a[:].rearrange("p n o -> p (n o)"))
                nc.scalar.copy(
                    out=dstb, in_=ps_b[:].rearrange("p n o -> p (n o)"))
            nc.scalar.dma_start(out=o_d[t, hc], in_=out_sb[:])

        NT = T_LOC * reps
        st = emit_loads(0, 0)
        for j in range(8):
            emit_xt_chunk(st, j)
        emit_scoresA(st)
        emit_ones_init()
        emit_zs(st)
        emit_invd2(st)
        emit_bcasts(st)
        emit_srts(st)
        for tt in range(NT):
            t = tt % T_LOC
            emit_e1T(st)
            # prefetch first 4 weight chunks before next-t input loads so the
            # final phase is never starved behind them on the queues
            wts = {hc: emit_wload(t, tt, hc) for hc in range(3)}
            nxt = None
            if tt + 1 < NT:
                nxt = emit_loads((tt + 1) % T_LOC, tt + 1)
                emit_scoresA(nxt)
            for hc in range(8):
                emit_final_chunk(t, tt, hc, *wts.pop(hc))
                if hc + 3 < 8:
                    wts[hc + 3] = emit_wload(t, tt, hc + 3)
                if nxt is not None:
                    emit_xt_chunk(nxt, hc)
                    if hc == 0:
                        emit_zs(nxt)
                    elif hc == 1:
                        emit_invd2(nxt)
                    elif hc == 3:
                        emit_bcasts(nxt)
                    elif hc == 5:
                        emit_srts(nxt)
            st = nxt

    nc.finalize()
    _CACHE[("nc", reps)] = nc
    return nc


def make_in_maps(inputs):
    import ml_dtypes
    bf16 = ml_dtypes.bfloat16
    f8e3 = ml_dtypes.float8_e3m4

    x = np.asarray(inputs["x"], dtype=np.float32)
    emb = np.asarray(inputs["dn_embeddings"], dtype=np.float32)
    w = np.asarray(inputs["weights_pool"], dtype=np.float32)
    bias = np.asarray(inputs["bias_pool"], dtype=np.float32)

    in_maps = []
    for c in range(NCORES):
        sl = slice(c * T_LOC, (c + 1) * T_LOC)
        xs = x[:, sl]  # [B, T_LOC, N, C]
        ws = w[sl]  # [T_LOC, N, K, C, O]
        # w0b rows 0:64 = (W0 - W2)^T(i,n,o), row 64 = bias
        w0b = np.empty((T_LOC, C + 1, N, O), np.float32)
        w0b[:, 0:C] = (ws[:, :, 0] - ws[:, :, 2]).transpose(0, 2, 1, 3)
        w0b[:, C] = bias[sl]
        # w12 rows 0:64 = W1^T, rows 64:128 = W2^T
        w12 = np.concatenate([
            ws[:, :, 1].transpose(0, 2, 1, 3),
            ws[:, :, 2].transpose(0, 2, 1, 3)], axis=1)
        in_maps.append({
            "xmbc_sh": np.ascontiguousarray(
                xs.transpose(1, 2, 0, 3)).astype(bf16),
            "xT_sh": np.ascontiguousarray(
                xs.transpose(1, 3, 0, 2)).astype(bf16),
            "emb_sh": np.ascontiguousarray(emb[sl]),
            "w0b_sh": np.ascontiguousarray(w0b).astype(bf16),
            "w12_sh": np.ascontiguousarray(w12).astype(f8e3),
            "ones_sh": np.ones((1, B * N), dtype=bf16),
        })
    return in_maps


def run_spmd(inputs, **kwargs):
    from concourse.bass_utils import run_bass_kernel_spmd

    nc = build_bass()
    in_maps = make_in_maps(inputs)
    res = run_bass_kernel_spmd(nc, in_maps, core_ids=list(range(NCORES)), **kwargs)
    outs = []
    for r in res.results:
        o2 = np.asarray(r["out_sh"]).reshape(T_LOC, 8, 4, B, 2, 8, O)
        # n = hc*64 + h*32 + g*8 + n8  ->  [B, T_LOC, hc, h, g, n8, O]
        o = o2.transpose(3, 0, 1, 4, 2, 5, 6).reshape(B, T_LOC, N, O)
        outs.append(o)
    out = np.concatenate(outs, axis=1)
    return out.astype(np.float32), res


def kernel(**inputs):
    out, _ = run_spmd(inputs)
    return out


# revision 42
# speedup vs baseline: 1.4610x; 1.0012x over previous
"""DAGCN kernel v9 for Trainium2, 8 NeuronCores, sharded over T (3 t/core).

Math per t (N=512 nodes, C=O=64, B=32, K=3):
  A    = relu(E E^T)  (rank-1 outer product, symmetric)
  PU   = exp(A) = max(exp(e_n e_m), 1)   (symmetric)
  Z_n  = sum_m PU[m, n]  (column sums == row sums by symmetry)
  S    = PU / Z  (row softmax);  d_n = S[n,n] = exp(e_n^2)/Z_n
  xg1  = S @ x;  xg2 = 2 d xg1 - x
  out  = x W0 + xg1 W1 + xg2 W2 + bias
       = x (W0 - W2) + xg1 W1 + (2 d xg1) W2 + bias     <- regrouped

Key structure vs the v3 baseline (185us HW):
  - Algebraic regroup removes the second message pass entirely: only
    xg1 = S@x runs on the PE (64 vs 128 N=512 matmuls per t); the third
    Chebyshev term xg1d = 2d*xg1 is an elementwise Pool-engine multiply
    against a broadcast tile.
  - No PE transposes for scores: PU is symmetric, so the message-pass
    moving operand srt[m,n] = PU[m,n] * inv[n] is a column-scale of the
    PU tile. Column vectors (inv, 2d) are broadcast across partitions
    with k=1 ones-row matmuls (the PE is the only partition broadcaster);
    column sums Z also come from k=1 ones-column matmuls.
  - Weights quantized by numeric class: [W1; W2] ship as fp8 e3m4 (they
    only multiply the small xg1/xg1d terms), [W0-W2; bias] stays bf16
    (it multiplies x = 99.5% of output variance). Mixed-dtype matmul
    (bf16 stationary x fp8e3 moving) measured absmax-rel 1.44e-2.
    Weight DMA drops from 12.06 to 8.06 MiB/t; W0-W2 precombined host-side.
  - Final contraction per n: 2 matmuls (k=65 bf16 [x;ones]x[W0';bias],
    k=128 fp8-moving [xg1;xg1d]x[W1;W2]) accumulating into psum packed as
    4x32-partition bands (band 3 in a separate bank: psum base 96 is an
    illegal matmul target), so one [128,512] copy moves 32 nodes and the
    output store runs at full 128-partition DMA rate.
  - Software-pipelined emission: scores(t+1) matmuls interleave between
    final(t) hc chunks; stack1 (x^T + ones row) is double-buffered by t
    parity so next-t x loads never wait on the running final phase.
    DMA queue discipline: loads never sit behind stores on any queue;
    weight chunks prefetch depth-3 on SP/SWDGE; x^T rides the scalar
    queue in [64, 4KB] chunks between output stores.
"""
import sys

sys.path.insert(0, "/opt/trn_rl_repo")
import numpy as np

CFG = ""

B, T, N, C, O, K = 32, 24, 512, 64, 64, 3
NCORES = 8
T_LOC = T // NCORES  # 3 time steps per core

_CACHE = {}


def build_bass(reps=1):
    if ("nc", reps) in _CACHE:
        return _CACHE[("nc", reps)]
    from contextlib import ExitStack

    import concourse.mybir as mybir
    from concourse import bacc
    import concourse.tile as tile
    from concourse.bass import ts

    f32 = mybir.dt.float32
    f32r = mybir.dt.float32r
    bf16 = mybir.dt.bfloat16
    f8e3 = mybir.dt.float8e3
    Alu = mybir.AluOpType
    Act = mybir.ActivationFunctionType

    nc = bacc.Bacc()
    xm_d = nc.dram_tensor("xmbc_sh", [T_LOC, N, B, C], bf16, kind="ExternalInput")
    xt_d = nc.dram_tensor("xT_sh", [T_LOC, C, B, N], bf16, kind="ExternalInput")
    e_d = nc.dram_tensor("emb_sh", [T_LOC, N], f32r, kind="ExternalInput")
    w0b_d = nc.dram_tensor("w0b_sh", [T_LOC, C + 1, N, O], bf16, kind="ExternalInput")
    w12_d = nc.dram_tensor("w12_sh", [T_LOC, 2 * C, N, O], f8e3, kind="ExternalInput")
    on_d = nc.dram_tensor("ones_sh", [1, B * N], bf16, kind="ExternalInput")
    # out packed for 128-partition DMA: [t, hc, g, b, h, n8, o]
    # n = hc*64 + h*32 + g*8 + n8; partitions = (g, b)
    o_d = nc.dram_tensor("out_sh", [T_LOC, 8, 128, 2, 8, O], bf16,
                         kind="ExternalOutput")

    with tile.TileContext(nc) as tc, ExitStack() as ctx:
        p1 = ctx.enter_context(tc.tile_pool(name="singles", bufs=1))
        p_row = ctx.enter_context(tc.tile_pool(name="rows", bufs=1))
        p_bc = ctx.enter_context(tc.tile_pool(name="bcast", bufs=2))
        p_pu = ctx.enter_context(tc.tile_pool(name="pu", bufs=4))
        p_srt = ctx.enter_context(tc.tile_pool(name="srt", bufs=5))
        p_x = ctx.enter_context(tc.tile_pool(name="xin", bufs=2))
        p_w0 = ctx.enter_context(tc.tile_pool(name="w0", bufs=4))
        p_w12 = ctx.enter_context(tc.tile_pool(name="w12", bufs=3))
        p_ob = ctx.enter_context(tc.tile_pool(name="ob", bufs=2))
        p_ps = ctx.enter_context(tc.tile_pool(name="ps", bufs=4, space="PSUM"))
        p_pso = ctx.enter_context(tc.tile_pool(name="pso", bufs=2, space="PSUM"))
        p_psob = ctx.enter_context(tc.tile_pool(name="psob", bufs=2, space="PSUM"))

        # persistent stacks:
        #   stack1 [65, B, N]  rows 0:64 x^T (c,b,n), row 64 ones (bias lane)
        #     double-buffered by t parity so next-t x loads never wait on the
        #     current final phase still reading it
        #   stack2 [128, B, N] rows 0:64 xg1^T, rows 64:128 xg1d^T
        stack1s = [p1.tile([C + 1, B, N], bf16, name=f"stk1_{i}", tag=f"s1_{i}")
                   for i in range(2)]
        stack2 = p1.tile([2 * C, B, N], bf16, name="stk2", tag="s2")
        # ones column [128, 1] for column-sum matmuls (k=1 trick needs a
        # [1, 128] stationary; ones row slice of on_d serves both)
        ones_col = p1.tile([128, 1], bf16, name="onec", tag="oc")
        nc.gpsimd.dma_start(out=ones_col, in_=on_d[0, 0:128].rearrange("(p f) -> p f", f=1))
        ones_row = p1.tile([1, 128], bf16, name="oner", tag="or")
        nc.gpsimd.dma_start(out=ones_row, in_=on_d[:, 0:128])

        def emit_ones_init():
            # stack1 row 64 = ones: split the [1, 32KB] single-partition DMA
            # into quarters on different queues; emitted after the critical
            # startup loads (only needed by the first final phase ~25us in)
            for i in range(2):
                for q, qeng in enumerate([nc.sync, nc.scalar, nc.gpsimd, nc.sync]):
                    qeng.dma_start(
                        out=stack1s[i][C:C + 1, 8 * q:8 * (q + 1), :].rearrange(
                            "p b n -> p (b n)"),
                        in_=on_d[:, 4096 * q:4096 * (q + 1)])

        # ---- software-pipelined emission ----
        # scores(t+1) is computed during final(t): PU matmuls right after
        # e1T(t); the cheap zs/bcast matmuls interleave between final-phase
        # hc chunks so the PE queue never stalls at a dependency head.
        PG = 8

        def emit_loads(t, tt):
            st = {"t": t, "tt": tt}
            e_row = p_row.tile([1, N], f32r, tag="erow")
            nc.scalar.dma_start(out=e_row, in_=e_d[t][None, :])
            st["e_row"] = e_row
            xmall = p_x.tile([128, 4, B, C], bf16, tag="xm")
            nc.sync.dma_start(
                out=xmall,
                in_=xm_d[t].rearrange("(mc p) b c -> p mc b c", p=128))
            st["xmall"] = xmall
            return st

        def emit_scoresA(st):
            e_row = st["e_row"]
            pus = []
            for mc in range(4):
                ps = p_ps.tile([128, N], f32, tag="big")
                nc.tensor.matmul(ps[:], e_row[:, ts(mc, 128)], e_row[:],
                                 start=True, stop=True)
                pu = p_pu.tile([128, N], bf16, tag="pu")
                nc.scalar.activation(pu[:], ps[:], Act.Exp)
                nc.vector.tensor_single_scalar(pu[:], pu[:], 1.0, Alu.max)
                pus.append(pu)
            st["pus"] = pus
            # row ops that only need e_row
            sq = p_row.tile([1, N], f32, tag="sq")
            nc.vector.tensor_mul(sq[:], e_row[:], e_row[:])
            esq = p_row.tile([1, N], f32, tag="esq")
            nc.scalar.activation(esq[:], sq[:], Act.Exp)
            st["esq"] = esq

        def emit_zs(st):
            zs_ps = p_ps.tile([128, N], f32, tag="big")
            for mc in range(4):
                nc.tensor.matmul(zs_ps[0:1, :], ones_col[:], st["pus"][mc][:],
                                 start=(mc == 0), stop=(mc == 3))
            st["zs_ps"] = zs_ps

        def emit_invd2(st):
            inv_row = p_row.tile([1, N], bf16, tag="invr")
            with nc.allow_low_precision(reason="inv feeds bf16 bcast matmul"):
                nc.vector.reciprocal(inv_row[:], st["zs_ps"][0:1, :])
            st["inv_row"] = inv_row
            t1 = p_row.tile([1, N], f32, tag="t1")
            nc.vector.tensor_tensor(out=t1[:], in0=st["esq"][:],
                                    in1=inv_row[:], op=Alu.mult)
            d2_row = p_row.tile([1, N], bf16, tag="d2r")
            nc.vector.tensor_single_scalar(d2_row[:], t1[:], 2.0, Alu.mult)
            st["d2_row"] = d2_row

        def emit_bcasts(st):
            invb_ps = p_ps.tile([128, N], f32, tag="big")
            nc.tensor.matmul(invb_ps[:], ones_row[:], st["inv_row"][:],
                             start=True, stop=True)
            inv_bc = p_bc.tile([128, N], bf16, tag="invbc")
            nc.vector.tensor_copy(out=inv_bc[:], in_=invb_ps[:])
            st["inv_bc"] = inv_bc
            d2b_ps = p_ps.tile([128, N], f32, tag="big")
            nc.tensor.matmul(d2b_ps[:], ones_row[:], st["d2_row"][:],
                             start=True, stop=True)
            d2_bc = p_bc.tile([128, N], bf16, tag="d2bc")
            nc.vector.tensor_copy(out=d2_bc[:], in_=d2b_ps[:])
            st["d2_bc"] = d2_bc

        def emit_srts(st):
            srts = []
            for mc in range(4):
                srt = p_srt.tile([128, N], bf16, tag="srt")
                nc.vector.tensor_tensor(out=srt[:], in0=st["pus"][mc][:],
                                        in1=st["inv_bc"][:], op=Alu.mult)
                srts.append(srt)
            st["srts"] = srts

        def emit_xt_chunk(st, j):
            # direct [64, 4KB] DMA of x^T rows for b-quarter j into the
            # parity stack, on the scalar queue where stores leave slack
            stk = stack1s[st["tt"] % 2]
            nc.scalar.dma_start(out=stk[0:C, 4 * j:4 * (j + 1), :],
                                in_=xt_d[st["t"], :, 4 * j:4 * (j + 1), :])

        def emit_e1T(st):
            xmall, srts = st["xmall"], st["srts"]
            for pr in range(16):  # b-pairs
                b0 = 2 * pr
                ps1 = p_ps.tile([128, N], f32, tag="big")
                for mc in range(4):
                    lhs = xmall[:, mc, b0:b0 + 2, :].rearrange("p b c -> p (b c)")
                    nc.tensor.matmul(ps1[:], lhs, srts[mc][:],
                                     start=(mc == 0), stop=(mc == 3))
                # xg1 psum->sbuf copies on DVE/Act (gpsimd cannot read PSUM);
                # xg1d = xg1 * d2 as sbuf-only TT on Pool engine
                ceng = [nc.scalar, nc.vector][pr % 2]
                if ceng is nc.scalar:
                    ceng.copy(out=stack2[0:C, b0, :], in_=ps1[0:C])
                else:
                    ceng.tensor_copy(out=stack2[0:C, b0, :], in_=ps1[0:C])
                ceng2 = [nc.vector, nc.scalar][pr % 2]
                if ceng2 is nc.scalar:
                    ceng2.copy(out=stack2[0:C, b0 + 1, :], in_=ps1[C:])
                else:
                    ceng2.tensor_copy(out=stack2[0:C, b0 + 1, :], in_=ps1[C:])
                nc.gpsimd.tensor_tensor(
                    out=stack2[C:, b0, :], in0=stack2[0:C, b0, :],
                    in1=st["d2_bc"][0:C], op=Alu.mult)
                nc.gpsimd.tensor_tensor(
                    out=stack2[C:, b0 + 1, :], in0=stack2[0:C, b0 + 1, :],
                    in1=st["d2_bc"][0:C], op=Alu.mult)

        def emit_wload(t, tt, hc):
            w0t = p_w0.tile([C + 1, 64, O], bf16, tag="w0", name=f"w0t_{tt}_{hc}")
            w12t = p_w12.tile([2 * C, 64, O], f8e3, tag="w12", name=f"w12t_{tt}_{hc}")
            nc.sync.dma_start(out=w0t, in_=w0b_d[t, :, ts(hc, 64), :])
            nc.gpsimd.dma_start(out=w12t, in_=w12_d[t, :, ts(hc, 64), :])
            return w0t, w12t

        def emit_final_chunk(t, tt, hc, w0t, w12t):
            # psum packed as 4 bands of 32 partitions; band 96 is illegal for
            # matmul (quadrant-3 HW bug): bands 0..2 in ps_a, band 3 in ps_b;
            # the copies shift band 3 to sbuf partitions 96:128.
            out_sb = p_ob.tile([128, 2, PG, O], bf16, tag="osb")
            for h in range(2):  # 32-n halves
                ps_a = p_pso.tile([96, PG, O], f32, tag="poa")
                ps_b = p_psob.tile([32, PG, O], f32, tag="pob")
                for g in range(4):  # partition bands
                    for j in range(PG):
                        nl = h * 32 + g * PG + j
                        ng = hc * 64 + nl
                        dst_ps = (ps_a[32 * g:32 * (g + 1), j, :]
                                  if g < 3 else ps_b[:, j, :])
                        nc.tensor.matmul(
                            dst_ps, stack1s[tt % 2][:, :, ng],
                            w0t[:, nl, :], start=True, stop=False)
                        nc.tensor.matmul(
                            dst_ps, stack2[:, :, ng],
                            w12t[:, nl, :], start=False, stop=True)
                dsta = out_sb[0:96, h, :, :].rearrange("p n o -> p (n o)")
                dstb = out_sb[96:128, h, :, :].rearrange("p n o -> p (n o)")
                nc.vector.tensor_copy(
                    out=dsta, in_=ps_a[:].rearrange("p n o -> p (n o)"))
                nc.scalar.copy(
                    out=dstb, in_=ps_b[:].rearrange("p n o -> p (n o)"))
            nc.scalar.dma_start(out=o_d[t, hc], in_=out_sb[:])

        NT = T_LOC * reps
        st = emit_loads(0, 0)
        for j in range(8):
            emit_xt_chunk(st, j)
        emit_scoresA(st)
        emit_ones_init()
        emit_zs(st)
        emit_invd2(st)
        emit_bcasts(st)
        emit_srts(st)
        for tt in range(NT):
            t = tt % T_LOC
            emit_e1T(st)
            # prefetch first 4 weight chunks before next-t input loads so the
            # final phase is never starved behind them on the queues
            wts = {hc: emit_wload(t, tt, hc) for hc in range(3)}
            nxt = None
            if tt + 1 < NT:
                nxt = emit_loads((tt + 1) % T_LOC, tt + 1)
                emit_scoresA(nxt)
            for hc in range(8):
                emit_final_chunk(t, tt, hc, *wts.pop(hc))
                if hc + 3 < 8:
                    wts[hc + 3] = emit_wload(t, tt, hc + 3)
                if nxt is not None:
                    emit_xt_chunk(nxt, hc)
                    if hc == 0:
                        emit_zs(nxt)
                    elif hc == 1:
                        emit_invd2(nxt)
                    elif hc == 3:
                        emit_bcasts(nxt)
                    elif hc == 5:
                        emit_srts(nxt)
            st = nxt

    nc.finalize()
    _CACHE[("nc", reps)] = nc
    return nc


def make_in_maps(inputs):
    import ml_dtypes
    bf16 = ml_dtypes.bfloat16
    f8e3 = ml_dtypes.float8_e3m4

    x = np.asarray(inputs["x"], dtype=np.float32)
    emb = np.asarray(inputs["dn_embeddings"], dtype=np.float32)
    w = np.asarray(inputs["weights_pool"], dtype=np.float32)
    bias = np.asarray(inputs["bias_pool"], dtype=np.float32)

    in_maps = []
    for c in range(NCORES):
        sl = slice(c * T_LOC, (c + 1) * T_LOC)
        xs = x[:, sl]  # [B, T_LOC, N, C]
        ws = w[sl]  # [T_LOC, N, K, C, O]
        # w0b rows 0:64 = (W0 - W2)^T(i,n,o), row 64 = bias
        w0b = np.empty((T_LOC, C + 1, N, O), np.float32)
        w0b[:, 0:C] = (ws[:, :, 0] - ws[:, :, 2]).transpose(0, 2, 1, 3)
        w0b[:, C] = bias[sl]
        # w12 rows 0:64 = W1^T, rows 64:128 = W2^T
        w12 = np.concatenate([
            ws[:, :, 1].transpose(0, 2, 1, 3),
            ws[:, :, 2].transpose(0, 2, 1, 3)], axis=1)
        in_maps.append({
            "xmbc_sh": np.ascontiguousarray(
                xs.transpose(1, 2, 0, 3)).astype(bf16),
            "xT_sh": np.ascontiguousarray(
                xs.transpose(1, 3, 0, 2)).astype(bf16),
            "emb_sh": np.ascontiguousarray(emb[sl]),
            "w0b_sh": np.ascontiguousarray(w0b).astype(bf16),
            "w12_sh": np.ascontiguousarray(w12).astype(f8e3),
            "ones_sh": np.ones((1, B * N), dtype=bf16),
        })
    return in_maps


def run_spmd(inputs, **kwargs):
    from concourse.bass_utils import run_bass_kernel_spmd

    nc = build_bass()
    in_maps = make_in_maps(inputs)
    res = run_bass_kernel_spmd(nc, in_maps, core_ids=list(range(NCORES)), **kwargs)
    outs = []
    for r in res.results:
        o2 = np.asarray(r["out_sh"]).reshape(T_LOC, 8, 4, B, 2, 8, O)
        # n = hc*64 + h*32 + g*8 + n8  ->  [B, T_LOC, hc, h, g, n8, O]
        o = o2.transpose(3, 0, 1, 4, 2, 5, 6).reshape(B, T_LOC, N, O)
        outs.append(o)
    out = np.concatenate(outs, axis=1)
    return out.astype(np.float32), res


def kernel(**inputs):
    out, _ = run_spmd(inputs)
    return out


# revision 43
# speedup vs baseline: 2.2569x; 1.5448x over previous
"""DAGCN kernel v9 for Trainium2, 8 NeuronCores, sharded over T (3 t/core).

Math per t (N=512 nodes, C=O=64, B=32, K=3):
  A    = relu(E E^T)  (rank-1 outer product, symmetric)
  PU   = exp(A) = max(exp(e_n e_m), 1)   (symmetric)
  Z_n  = sum_m PU[m, n]  (column sums == row sums by symmetry)
  S    = PU / Z  (row softmax);  d_n = S[n,n] = exp(e_n^2)/Z_n
  xg1  = S @ x;  xg2 = 2 d xg1 - x
  out  = x W0 + xg1 W1 + xg2 W2 + bias
       = x (W0 - W2) + xg1 W1 + (2 d xg1) W2 + bias     <- regrouped

Key structure vs the v3 baseline (185us HW):
  - Algebraic regroup removes the second message pass entirely: only
    xg1 = S@x runs on the PE (64 vs 128 N=512 matmuls per t); the third
    Chebyshev term xg1d = 2d*xg1 is an elementwise Pool-engine multiply
    against a broadcast tile.
  - No PE transposes for scores: PU is symmetric, so the message-pass
    moving operand srt[m,n] = PU[m,n] * inv[n] is a column-scale of the
    PU tile. Column vectors (inv, 2d) are broadcast across partitions
    with k=1 ones-row matmuls (the PE is the only partition broadcaster);
    column sums Z also come from k=1 ones-column matmuls.
  - Weights quantized by numeric class: [W1; W2] ship as fp8 e3m4 (they
    only multiply the small xg1/xg1d terms), [W0-W2; bias] stays bf16
    (it multiplies x = 99.5% of output variance). Mixed-dtype matmul
    (bf16 stationary x fp8e3 moving) measured absmax-rel 1.44e-2.
    Weight DMA drops from 12.06 to 8.06 MiB/t; W0-W2 precombined host-side.
  - Final contraction per n: 2 matmuls (k=65 bf16 [x;ones]x[W0';bias],
    k=128 fp8-moving [xg1;xg1d]x[W1;W2]) accumulating into psum packed as
    4x32-partition bands (band 3 in a separate bank: psum base 96 is an
    illegal matmul target), so one [128,512] copy moves 32 nodes and the
    output store runs at full 128-partition DMA rate.
  - Software-pipelined emission: scores(t+1) matmuls interleave between
    final(t) hc chunks; stack1 (x^T + ones row) is double-buffered by t
    parity so next-t x loads never wait on the running final phase.
    DMA queue discipline: loads never sit behind stores on any queue;
    weight chunks prefetch depth-3 on SP/SWDGE; x^T rides the scalar
    queue in [64, 4KB] chunks between output stores.
"""
import sys

sys.path.insert(0, "/opt/trn_rl_repo")
import numpy as np

CFG = ""

B, T, N, C, O, K = 32, 24, 512, 64, 64, 3
NCORES = 8
T_LOC = T // NCORES  # 3 time steps per core

_CACHE = {}


def build_bass(reps=1):
    if ("nc", reps) in _CACHE:
        return _CACHE[("nc", reps)]
    from contextlib import ExitStack

    import concourse.mybir as mybir
    from concourse import bacc
    import concourse.tile as tile
    from concourse.bass import ts

    f32 = mybir.dt.float32
    f32r = mybir.dt.float32r
    bf16 = mybir.dt.bfloat16
    f8e3 = mybir.dt.float8e3
    Alu = mybir.AluOpType
    Act = mybir.ActivationFunctionType

    nc = bacc.Bacc()
    xm_d = nc.dram_tensor("xmbc_sh", [T_LOC, N, B, C], bf16, kind="ExternalInput")
    xt_d = nc.dram_tensor("xT_sh", [T_LOC, C, B, N], bf16, kind="ExternalInput")
    e_d = nc.dram_tensor("emb_sh", [T_LOC, N], f32r, kind="ExternalInput")
    w0b_d = nc.dram_tensor("w0b_sh", [T_LOC, C + 1, N, O], bf16, kind="ExternalInput")
    w12_d = nc.dram_tensor("w12_sh", [T_LOC, 2 * C, N, O], f8e3, kind="ExternalInput")
    on_d = nc.dram_tensor("ones_sh", [1, B * N], bf16, kind="ExternalInput")
    # out packed for 128-partition DMA: [t, hc, g, b, h, n8, o]
    # n = hc*64 + h*32 + g*8 + n8; partitions = (g, b)
    o_d = nc.dram_tensor("out_sh", [T_LOC, 8, 128, 2, 8, O], bf16,
                         kind="ExternalOutput")

    with tile.TileContext(nc) as tc, ExitStack() as ctx:
        p1 = ctx.enter_context(tc.tile_pool(name="singles", bufs=1))
        p_row = ctx.enter_context(tc.tile_pool(name="rows", bufs=1))
        p_bc = ctx.enter_context(tc.tile_pool(name="bcast", bufs=2))
        p_pu = ctx.enter_context(tc.tile_pool(name="pu", bufs=4))
        p_srt = ctx.enter_context(tc.tile_pool(name="srt", bufs=5))
        p_x = ctx.enter_context(tc.tile_pool(name="xin", bufs=2))
        p_w0 = ctx.enter_context(tc.tile_pool(name="w0", bufs=4))
        p_w12 = ctx.enter_context(tc.tile_pool(name="w12", bufs=3))
        p_ob = ctx.enter_context(tc.tile_pool(name="ob", bufs=2))
        p_ps = ctx.enter_context(tc.tile_pool(name="ps", bufs=4, space="PSUM"))
        p_pso = ctx.enter_context(tc.tile_pool(name="pso", bufs=2, space="PSUM"))
        p_psob = ctx.enter_context(tc.tile_pool(name="psob", bufs=2, space="PSUM"))

        # persistent stacks:
        #   stack1 [65, B, N]  rows 0:64 x^T (c,b,n), row 64 ones (bias lane)
        #     double-buffered by t parity so next-t x loads never wait on the
        #     current final phase still reading it
        #   stack2 [128, B, N] rows 0:64 xg1^T, rows 64:128 xg1d^T
        stack1s = [p1.tile([C + 1, B, N], bf16, name=f"stk1_{i}", tag=f"s1_{i}")
                   for i in range(2)]
        stack2 = p1.tile([2 * C, B, N], bf16, name="stk2", tag="s2")
        # ones column [128, 1] for column-sum matmuls (k=1 trick needs a
        # [1, 128] stationary; ones row slice of on_d serves both)
        ones_col = p1.tile([128, 1], bf16, name="onec", tag="oc")
        nc.gpsimd.dma_start(out=ones_col, in_=on_d[0, 0:128].rearrange("(p f) -> p f", f=1))
        ones_row = p1.tile([1, 128], bf16, name="oner", tag="or")
        nc.gpsimd.dma_start(out=ones_row, in_=on_d[:, 0:128])

        def emit_ones_init():
            # stack1 row 64 = ones: split the [1, 32KB] single-partition DMA
            # into quarters on different queues; emitted after the critical
            # startup loads (only needed by the first final phase ~25us in)
            for i in range(2):
                for q, qeng in enumerate([nc.sync, nc.scalar, nc.gpsimd, nc.sync]):
                    qeng.dma_start(
                        out=stack1s[i][C:C + 1, 8 * q:8 * (q + 1), :].rearrange(
                            "p b n -> p (b n)"),
                        in_=on_d[:, 4096 * q:4096 * (q + 1)])

        # ---- software-pipelined emission ----
        # scores(t+1) is computed during final(t): PU matmuls right after
        # e1T(t); the cheap zs/bcast matmuls interleave between final-phase
        # hc chunks so the PE queue never stalls at a dependency head.
        PG = 8

        def emit_loads(t, tt):
            st = {"t": t, "tt": tt}
            e_row = p_row.tile([1, N], f32r, tag="erow")
            nc.scalar.dma_start(out=e_row, in_=e_d[t][None, :])
            st["e_row"] = e_row
            xmall = p_x.tile([128, 4, B, C], bf16, tag="xm")
            nc.sync.dma_start(
                out=xmall,
                in_=xm_d[t].rearrange("(mc p) b c -> p mc b c", p=128))
            st["xmall"] = xmall
            return st

        def emit_scoresA(st):
            e_row = st["e_row"]
            pus = []
            for mc in range(4):
                ps = p_ps.tile([128, N], f32, tag="big")
                nc.tensor.matmul(ps[:], e_row[:, ts(mc, 128)], e_row[:],
                                 start=True, stop=True)
                pu = p_pu.tile([128, N], bf16, tag="pu")
                nc.scalar.activation(pu[:], ps[:], Act.Exp)
                nc.vector.tensor_single_scalar(pu[:], pu[:], 1.0, Alu.max)
                pus.append(pu)
            st["pus"] = pus
            # row ops that only need e_row
            sq = p_row.tile([1, N], f32, tag="sq")
            nc.vector.tensor_mul(sq[:], e_row[:], e_row[:])
            esq = p_row.tile([1, N], f32, tag="esq")
            nc.scalar.activation(esq[:], sq[:], Act.Exp)
            st["esq"] = esq

        def emit_zs(st):
            zs_ps = p_ps.tile([128, N], f32, tag="big")
            for mc in range(4):
                nc.tensor.matmul(zs_ps[0:1, :], ones_col[:], st["pus"][mc][:],
                                 start=(mc == 0), stop=(mc == 3))
            st["zs_ps"] = zs_ps

        def emit_invd2(st):
            inv_row = p_row.tile([1, N], bf16, tag="invr")
            with nc.allow_low_precision(reason="inv feeds bf16 bcast matmul"):
                nc.vector.reciprocal(inv_row[:], st["zs_ps"][0:1, :])
            st["inv_row"] = inv_row
            t1 = p_row.tile([1, N], f32, tag="t1")
            nc.vector.tensor_tensor(out=t1[:], in0=st["esq"][:],
                                    in1=inv_row[:], op=Alu.mult)
            d2_row = p_row.tile([1, N], bf16, tag="d2r")
            nc.vector.tensor_single_scalar(d2_row[:], t1[:], 2.0, Alu.mult)
            st["d2_row"] = d2_row

        def emit_bcasts(st):
            invb_ps = p_ps.tile([128, N], f32, tag="big")
            nc.tensor.matmul(invb_ps[:], ones_row[:], st["inv_row"][:],
                             start=True, stop=True)
            inv_bc = p_bc.tile([128, N], bf16, tag="invbc")
            nc.vector.tensor_copy(out=inv_bc[:], in_=invb_ps[:])
            st["inv_bc"] = inv_bc
            d2b_ps = p_ps.tile([128, N], f32, tag="big")
            nc.tensor.matmul(d2b_ps[:], ones_row[:], st["d2_row"][:],
                             start=True, stop=True)
            d2_bc = p_bc.tile([128, N], bf16, tag="d2bc")
            nc.vector.tensor_copy(out=d2_bc[:], in_=d2b_ps[:])
            st["d2_bc"] = d2_bc

        def emit_srts(st):
            # sbuf-only TT: run on Pool, which idles during the final phase,
            # so the srts never queue behind DVE out-copies
            srts = []
            for mc in range(4):
                srt = p_srt.tile([128, N], bf16, tag="srt")
                nc.gpsimd.tensor_tensor(out=srt[:], in0=st["pus"][mc][:],
                                        in1=st["inv_bc"][:], op=Alu.mult)
                srts.append(srt)
            st["srts"] = srts

        def emit_xt_chunk(st, j):
            # direct [64, 4KB] DMA of x^T rows for b-quarter j into the
            # parity stack, on the scalar queue where stores leave slack
            stk = stack1s[st["tt"] % 2]
            nc.scalar.dma_start(out=stk[0:C, 4 * j:4 * (j + 1), :],
                                in_=xt_d[st["t"], :, 4 * j:4 * (j + 1), :])

        def emit_e1T(st):
            xmall, srts = st["xmall"], st["srts"]
            for pr in range(16):  # b-pairs
                b0 = 2 * pr
                ps1 = p_ps.tile([128, N], f32, tag="big")
                for mc in range(4):
                    lhs = xmall[:, mc, b0:b0 + 2, :].rearrange("p b c -> p (b c)")
                    nc.tensor.matmul(ps1[:], lhs, srts[mc][:],
                                     start=(mc == 0), stop=(mc == 3))
                # xg1 psum->sbuf copies on DVE/Act (gpsimd cannot read PSUM);
                # xg1d = xg1 * d2 as sbuf-only TT on Pool engine
                ceng = [nc.scalar, nc.vector][pr % 2]
                if ceng is nc.scalar:
                    ceng.copy(out=stack2[0:C, b0, :], in_=ps1[0:C])
                else:
                    ceng.tensor_copy(out=stack2[0:C, b0, :], in_=ps1[0:C])
                ceng2 = [nc.vector, nc.scalar][pr % 2]
                if ceng2 is nc.scalar:
                    ceng2.copy(out=stack2[0:C, b0 + 1, :], in_=ps1[C:])
                else:
                    ceng2.tensor_copy(out=stack2[0:C, b0 + 1, :], in_=ps1[C:])
                nc.gpsimd.tensor_tensor(
                    out=stack2[C:, b0, :], in0=stack2[0:C, b0, :],
                    in1=st["d2_bc"][0:C], op=Alu.mult)
                nc.gpsimd.tensor_tensor(
                    out=stack2[C:, b0 + 1, :], in0=stack2[0:C, b0 + 1, :],
                    in1=st["d2_bc"][0:C], op=Alu.mult)

        def emit_wload(t, tt, hc):
            w0t = p_w0.tile([C + 1, 64, O], bf16, tag="w0", name=f"w0t_{tt}_{hc}")
            w12t = p_w12.tile([2 * C, 64, O], f8e3, tag="w12", name=f"w12t_{tt}_{hc}")
            nc.sync.dma_start(out=w0t, in_=w0b_d[t, :, ts(hc, 64), :])
            nc.gpsimd.dma_start(out=w12t, in_=w12_d[t, :, ts(hc, 64), :])
            return w0t, w12t

        def emit_final_chunk(t, tt, hc, w0t, w12t):
            # psum packed as 4 bands of 32 partitions; band 96 is illegal for
            # matmul (quadrant-3 HW bug): bands 0..2 in ps_a, band 3 in ps_b;
            # the copies shift band 3 to sbuf partitions 96:128.
            out_sb = p_ob.tile([128, 2, PG, O], bf16, tag="osb")
            for h in range(2):  # 32-n halves
                ps_a = p_pso.tile([96, PG, O], f32, tag="poa")
                ps_b = p_psob.tile([32, PG, O], f32, tag="pob")
                for g in range(4):  # partition bands
                    for j in range(PG):
                        nl = h * 32 + g * PG + j
                        ng = hc * 64 + nl
                        dst_ps = (ps_a[32 * g:32 * (g + 1), j, :]
                                  if g < 3 else ps_b[:, j, :])
                        nc.tensor.matmul(
                            dst_ps, stack1s[tt % 2][:, :, ng],
                            w0t[:, nl, :], start=True, stop=False)
                        nc.tensor.matmul(
                            dst_ps, stack2[:, :, ng],
                            w12t[:, nl, :], start=False, stop=True)
                dsta = out_sb[0:96, h, :, :].rearrange("p n o -> p (n o)")
                dstb = out_sb[96:128, h, :, :].rearrange("p n o -> p (n o)")
                nc.vector.tensor_copy(
                    out=dsta, in_=ps_a[:].rearrange("p n o -> p (n o)"))
                nc.scalar.copy(
                    out=dstb, in_=ps_b[:].rearrange("p n o -> p (n o)"))
            nc.scalar.dma_start(out=o_d[t, hc], in_=out_sb[:])

        NT = T_LOC * reps
        st = emit_loads(0, 0)
        for j in range(8):
            emit_xt_chunk(st, j)
        emit_scoresA(st)
        emit_ones_init()
        emit_zs(st)
        emit_invd2(st)
        emit_bcasts(st)
        emit_srts(st)
        for tt in range(NT):
            t = tt % T_LOC
            emit_e1T(st)
            # prefetch first 4 weight chunks before next-t input loads so the
            # final phase is never starved behind them on the queues
            wts = {hc: emit_wload(t, tt, hc) for hc in range(3)}
            nxt = None
            if tt + 1 < NT:
                nxt = emit_loads((tt + 1) % T_LOC, tt + 1)
                emit_scoresA(nxt)
            for hc in range(8):
                emit_final_chunk(t, tt, hc, *wts.pop(hc))
                if hc + 3 < 8:
                    wts[hc + 3] = emit_wload(t, tt, hc + 3)
                if nxt is not None:
                    emit_xt_chunk(nxt, hc)
                    if hc == 0:
                        emit_zs(nxt)
                    elif hc == 1:
                        emit_invd2(nxt)
                    elif hc == 2:
                        emit_bcasts(nxt)
                    elif hc == 4:
                        emit_srts(nxt)
            st = nxt

    nc.finalize()
    _CACHE[("nc", reps)] = nc
    return nc


def make_in_maps(inputs):
    import ml_dtypes
    bf16 = ml_dtypes.bfloat16
    f8e3 = ml_dtypes.float8_e3m4

    x = np.asarray(inputs["x"], dtype=np.float32)
    emb = np.asarray(inputs["dn_embeddings"], dtype=np.float32)
    w = np.asarray(inputs["weights_pool"], dtype=np.float32)
    bias = np.asarray(inputs["bias_pool"], dtype=np.float32)

    in_maps = []
    for c in range(NCORES):
        sl = slice(c * T_LOC, (c + 1) * T_LOC)
        xs = x[:, sl]  # [B, T_LOC, N, C]
        ws = w[sl]  # [T_LOC, N, K, C, O]
        # w0b rows 0:64 = (W0 - W2)^T(i,n,o), row 64 = bias
        w0b = np.empty((T_LOC, C + 1, N, O), np.float32)
        w0b[:, 0:C] = (ws[:, :, 0] - ws[:, :, 2]).transpose(0, 2, 1, 3)
        w0b[:, C] = bias[sl]
        # w12 rows 0:64 = W1^T, rows 64:128 = W2^T
        w12 = np.concatenate([
            ws[:, :, 1].transpose(0, 2, 1, 3),
            ws[:, :, 2].transpose(0, 2, 1, 3)], axis=1)
        in_maps.append({
            "xmbc_sh": np.ascontiguousarray(
                xs.transpose(1, 2, 0, 3)).astype(bf16),
            "xT_sh": np.ascontiguousarray(
                xs.transpose(1, 3, 0, 2)).astype(bf16),
            "emb_sh": np.ascontiguousarray(emb[sl]),
            "w0b_sh": np.ascontiguousarray(w0b).astype(bf16),
            "w12_sh": np.ascontiguousarray(w12).astype(f8e3),
            "ones_sh": np.ones((1, B * N), dtype=bf16),
        })
    return in_maps


def run_spmd(inputs, **kwargs):
    from concourse.bass_utils import run_bass_kernel_spmd

    nc = build_bass()
    in_maps = make_in_maps(inputs)
    res = run_bass_kernel_spmd(nc, in_maps, core_ids=list(range(NCORES)), **kwargs)
    outs = []
    for r in res.results:
        o2 = np.asarray(r["out_sh"]).reshape(T_LOC, 8, 4, B, 2, 8, O)
        # n = hc*64 + h*32 + g*8 + n8  ->  [B, T_LOC, hc, h, g, n8, O]
        o = o2.transpose(3, 0, 1, 4, 2, 5, 6).reshape(B, T_LOC, N, O)
        outs.append(o)
    out = np.concatenate(outs, axis=1)
    return out.astype(np.float32), res


def kernel(**inputs):
    out, _ = run_spmd(inputs)
    return out
